# revision 12
# baseline (speedup 1.0000x reference)
"""ColorDenseCRFLoss on 8 Trainium2 NeuronCores.

Math: loss = -W/N * sum_n sum_ij K_ij S_ij, where for each image n
  K_ij = exp(-0.5*||f_i - f_j||^2)   (f = nearest-downsampled RGB / 15, P=4096 pts)
  S_ij = sum_k seg_k,i seg_k,j       (seg = bilinear-downsampled softmax, K=21)
Bilinear downsample at exactly 2x == 2x2 average pooling; nearest == stride-2.

Sharding: 2 cores per image (batch N=4 -> 8 cores). Symmetry of K and S is
exploited with a circulant block scheme: core h of image n owns 16 of the 32
row-blocks (I = 16h..16h+15) and for each row-block I computes column blocks
J = I..I+16 (mod 32, d=0..16), weighting d=0 and d=16 blocks by 1/2 (applied
as an exp() bias of -ln2); the grand total is then doubled. Each unordered
block pair is thus counted exactly once per symmetric half. The mod-32 wrap
is made contiguous by handing each core its inputs rotated by 2048*h points
(pure host-side reindexing = np.roll of image rows).

On-device per core: G = A'^T B' via PE (split-bf16 augmented features, K=15,
exact to ~1e-3 abs), K = exp(G) via ACT from PSUM, S-block via PE from the
2x2-summed seg (bf16, pooled on-device with DVE adds), then a fused DVE
tensor_tensor_reduce accumulates sum(K*S). Host sums the 8 per-core partials
(the all-reduce) and applies constants (x2 symmetry, /16 pool scale, -W/N).
"""

import sys

for _p in ("/opt/trn_rl_repo",):
    if _p not in sys.path:
        sys.path.insert(0, _p)

import numpy as np
import ml_dtypes

import bass_rust
import concourse.bass as bass
import concourse.mybir as mybir
from concourse.tile import TileContext
from concourse.bass_utils import run_bass_kernel_spmd

F32 = mybir.dt.float32
BF16 = mybir.dt.bfloat16
LN2 = 0.6931471805599453

WEIGHT = 1e-7
SIGMA_RGB = 15.0
N_IMG = 4
P = 4096          # 64*64 points per image
NB = 32           # 128-point blocks per image
VB = 16           # row-blocks per core
WIN = 17 * 128    # d = 0..16 column window
G_CHUNKS = [(0, 512), (512, 1024), (1024, 1536), (1536, 2048), (2048, 2176)]

_CACHED = {}


def _build_nc():
    nc = bass.Bass(trn_type="TRN2", target_bir_lowering=False, debug=False)
    seg_d = nc.dram_tensor("segr", [21, 128, 128], F32, kind="ExternalInput")
    a_d = nc.dram_tensor("abf", [15, 2048], BF16, kind="ExternalInput")
    b_d = nc.dram_tensor("bbf", [15, 4096], BF16, kind="ExternalInput")
    out_d = nc.dram_tensor("acc", [128, 1], F32, kind="ExternalOutput")

    EXP = mybir.ActivationFunctionType.Exp
    MULT = mybir.AluOpType.mult
    ADD = mybir.AluOpType.add

    with TileContext(nc) as tc:
        with (
            tc.tile_pool(name="const", bufs=1) as constp,
            tc.tile_pool(name="pre", bufs=1) as prep,
            tc.tile_pool(name="kbuf", bufs=80) as kp,
            tc.tile_pool(name="scr", bufs=2) as scp,
            tc.tile_pool(name="pg", bufs=3, space="PSUM") as pg,
            tc.tile_pool(name="ps", bufs=3, space="PSUM") as ps,
        ):
            bias0 = constp.tile([128, 1], F32, tag="bias0")
            nc.vector.memset(bias0[:], 0.0)
            biasH = constp.tile([128, 1], F32, tag="biasH")
            nc.vector.memset(biasH[:], -LN2)

            # Warm-up exp: anchors the one-time ACT table load here (off the
            # critical path, and keeps later activations at <=2 sync waits).
            warm = constp.tile([128, 1], F32, tag="warm")
            nc.scalar.activation(warm[:], biasH[:], EXP, bias=bias0[:])

            abf = constp.tile([15, 2048], BF16)
            nc.sync.dma_start(abf[:], a_d.ap())
            bbf = constp.tile([15, 4096], BF16)
            nc.sync.dma_start(bbf[:], b_d.ap())

            # 2x2 pooling of seg (unscaled sum; the /4 per factor is folded
            # into the host-side constant). Four strided quadrant loads in a
            # [84, 1024] layout (k spans 4 aligned partitions), summed, then one SBUF->SBUF
            # DMA re-layout to the matmul layout [21, 4096] (1024 | 4096).
            segap = seg_d.ap()
            quads = [
                segap[:, 0::2, 0::2],
                segap[:, 0::2, 1::2],
                segap[:, 1::2, 0::2],
                segap[:, 1::2, 1::2],
            ]
            qt = []
            for qi, q in enumerate(quads):
                t = prep.tile([84, 1024], F32, tag=f"q{qi}")
                nc.sync.dma_start(t[:], q)
                qt.append(t)
            s01 = prep.tile([84, 1024], F32, tag="s01")
            nc.vector.tensor_add(s01[:], qt[0][:], qt[1][:])
            s23 = prep.tile([84, 1024], F32, tag="s23")
            nc.vector.tensor_add(s23[:], qt[2][:], qt[3][:])
            pooled_flat = prep.tile([84, 1024], BF16, tag="poolf")
            nc.vector.tensor_add(pooled_flat[:], s01[:], s23[:])
            seg_sb = prep.tile([21, 4096], BF16, tag="segsb")
            nc.sync.dma_start(seg_sb[:], pooled_flat[:])

            accT = constp.tile([128, 80], F32)

            idx = 0
            for v in range(VB):
                base = v * 128
                ga = abf[:, base : base + 128]
                sa = seg_sb[:, base : base + 128]
                for c0, c1 in G_CHUNKS:
                    w = c1 - c0
                    pGt = pg.tile([128, 512], F32, tag="pg")
                    nc.tensor.matmul(
                        pGt[:, :w],
                        ga,
                        bbf[:, base + c0 : base + c1],
                        start=True,
                        stop=True,
                    )
                    kt = kp.tile([128, 512], BF16, tag="k")
                    if c0 == 0:
                        # d=0 (diagonal) block weighted 1/2 via exp bias
                        nc.scalar.activation(kt[:, 0:128], pGt[:, 0:128], EXP, bias=biasH[:])
                        nc.scalar.activation(kt[:, 128:512], pGt[:, 128:512], EXP, bias=bias0[:])
                    elif c0 == 2048:
                        # d=16 block weighted 1/2
                        nc.scalar.activation(kt[:, :w], pGt[:, :w], EXP, bias=biasH[:, :])
                    else:
                        nc.scalar.activation(kt[:, :w], pGt[:, :w], EXP, bias=bias0[:, :])
                    pSt = ps.tile([128, 512], F32, tag="ps")
                    nc.tensor.matmul(
                        pSt[:, :w],
                        sa,
                        seg_sb[:, base + c0 : base + c1],
                        start=True,
                        stop=True,
                    )
                    sct = scp.tile([128, 512], F32, tag="sc")
                    # out = (K mult 1.0) mult S; accum_out = sum(out) per row
                    nc.vector.scalar_tensor_tensor(
                        out=sct[:, :w],
                        in0=kt[:, :w],
                        scalar=1.0,
                        in1=pSt[:, :w],
                        op0=MULT,
                        op1=MULT,
                        accum_out=accT[:, idx : idx + 1],
                    )
                    idx += 1

            red = scp.tile([128, 1], F32, tag="red")
            nc.vector.tensor_reduce(
                red[:], accT[:], axis=mybir.AxisListType.X, op=ADD
            )
            nc.sync.dma_start(out_d.ap(), red[:])
    _split_multiwait(nc)
    return nc


def _split_multiwait(nc):
    """The walrus build here encodes at most one semaphore wait per compute
    instruction (setupSyncWait: 'Too many sync wait commands'). Tile emits
    multi-wait instructions, so hoist all but one wait onto standalone
    EventSemaphore instructions (what raw-bass wait_ge emits) placed just
    before the instruction on the same engine queue. Semantics identical:
    the engine blocks on each wait in turn."""
    ctr = 0
    for f in nc.m.functions:
        for blk in f.blocks:
            insts = blk.instructions
            out = []
            for inst in insts:
                si = inst.sync_info
                if si is not None and len(si.on_wait) > 1:
                    waits = list(si.on_wait)
                    for w in waits[:-1]:
                        es = mybir.InstEventSemaphore(
                            name=f"WSPLIT-{ctr}", ins=[], outs=[]
                        )
                        ctr += 1
                        es.engine = inst.engine
                        es.sync_info = bass_rust.SyncInfo(on_wait=[w], on_update=[])
                        out.append(es)
                    inst.sync_info = bass_rust.SyncInfo(
                        on_wait=[waits[-1]], on_update=list(si.on_update)
                    )
                out.append(inst)
            insts[:] = out


def _host_prep(images, segmentations):
    """Per-core inputs. Host work is reindexing (roll/stride/reshape) plus the
    tiny [5,4096] feature augmentation; all seg arithmetic happens on-device."""
    bf = ml_dtypes.bfloat16
    in_maps = []
    for c in range(8):
        n, h = c // 2, c % 2
        img = images[n][:, ::2, ::2]                       # nearest resize
        img = np.roll(img, -32 * h, axis=1).reshape(3, P)  # circulant rotation
        f = (img / SIGMA_RGB).astype(np.float32)
        f = f - f.mean(axis=1, keepdims=True)              # d2-invariant centering
        sq = (f * f).sum(axis=0)
        ones = np.ones((1, P), np.float32)
        b5 = np.concatenate([f, ones, (-0.5 * sq)[None]], axis=0)
        a5 = np.concatenate([f, (-0.5 * sq)[None], ones], axis=0)[:, : P // 2]

        def split(x):
            hi = x.astype(bf)
            lo = (x - hi.astype(np.float32)).astype(bf)
            return hi, lo

        a5h, a5l = split(a5)
        b5h, b5l = split(b5)
        abf = np.concatenate([a5h, a5l, a5h], axis=0)      # [15, 2048] bf16
        bbf = np.concatenate([b5h, b5h, b5l], axis=0)      # [15, 4096] bf16
        segr = np.roll(segmentations[n], -64 * h, axis=1)  # [21,128,128] f32
        in_maps.append(
            {
                "segr": np.ascontiguousarray(segr, dtype=np.float32),
                "abf": np.ascontiguousarray(abf),
                "bbf": np.ascontiguousarray(bbf),
            }
        )
    return in_maps


def run(images, segmentations, trace=False):
    if "nc" not in _CACHED:
        _CACHED["nc"] = _build_nc()
    nc = _CACHED["nc"]
    in_maps = _host_prep(np.asarray(images), np.asarray(segmentations))
    res = run_bass_kernel_spmd(nc, in_maps, list(range(8)), trace=trace)
    total = np.float64(0.0)
    for r in res.results:
        total += np.float64(r["acc"].astype(np.float64).sum())
    # x2 symmetric halves, /16 unscaled 2x2 pool (quadratic), -W, /N batch mean
    loss = -WEIGHT * 2.0 * total / 16.0 / N_IMG
    return np.array([loss], dtype=np.float32), res


def kernel(images, segmentations):
    out, _ = run(images, segmentations, trace=False)
    return out


# revision 13
# speedup vs baseline: 3.4461x; 3.4461x over previous
"""ColorDenseCRFLoss on 8 Trainium2 NeuronCores.

Math: loss = -W/N * sum_n sum_ij K_ij S_ij, where for each image n
  K_ij = exp(-0.5*||f_i - f_j||^2)   (f = nearest-downsampled RGB / 15, P=4096 pts)
  S_ij = sum_k seg_k,i seg_k,j       (seg = bilinear-downsampled softmax, K=21)
Bilinear downsample at exactly 2x == 2x2 average pooling; nearest == stride-2.

Sharding: 2 cores per image (batch N=4 -> 8 cores). Symmetry of K and S is
exploited with a circulant block scheme: core h of image n owns 16 of the 32
row-blocks (I = 16h..16h+15) and for each row-block I computes column blocks
J = I..I+16 (mod 32, d=0..16), weighting d=0 and d=16 blocks by 1/2 (applied
as an exp() bias of -ln2); the grand total is then doubled. Each unordered
block pair is thus counted exactly once per symmetric half. The mod-32 wrap
is made contiguous by handing each core its inputs rotated by 2048*h points
(pure host-side reindexing = np.roll of image rows).

On-device per core: G = A'^T B' via PE (split-bf16 augmented features, K=15,
exact to ~1e-3 abs), K = exp(G) via ACT from PSUM, S-block via PE from the
2x2-summed seg (bf16, pooled on-device with DVE adds), then a fused DVE
tensor_tensor_reduce accumulates sum(K*S). Host sums the 8 per-core partials
(the all-reduce) and applies constants (x2 symmetry, /16 pool scale, -W/N).
"""

import sys

for _p in ("/opt/trn_rl_repo",):
    if _p not in sys.path:
        sys.path.insert(0, _p)

import numpy as np
import ml_dtypes

import bass_rust
import concourse.bass as bass
import concourse.mybir as mybir
from concourse.tile import TileContext
from concourse.bass_utils import run_bass_kernel_spmd

F32 = mybir.dt.float32
BF16 = mybir.dt.bfloat16
LN2 = 0.6931471805599453

WEIGHT = 1e-7
SIGMA_RGB = 15.0
N_IMG = 4
P = 4096          # 64*64 points per image
NB = 32           # 128-point blocks per image
VB = 16           # row-blocks per core
WIN = 17 * 128    # d = 0..16 column window
G_CHUNKS = [(0, 512), (512, 1024), (1024, 1536), (1536, 2048), (2048, 2176)]

_CACHED = {}


def _build_nc():
    nc = bass.Bass(trn_type="TRN2", target_bir_lowering=False, debug=False)
    seg_d = nc.dram_tensor("segr", [21, 128, 128], F32, kind="ExternalInput")
    a_d = nc.dram_tensor("abf", [15, 2048], BF16, kind="ExternalInput")
    b_d = nc.dram_tensor("bbf", [15, 4096], BF16, kind="ExternalInput")
    out_d = nc.dram_tensor("acc", [128, 1], F32, kind="ExternalOutput")

    EXP = mybir.ActivationFunctionType.Exp
    MULT = mybir.AluOpType.mult
    ADD = mybir.AluOpType.add

    with TileContext(nc) as tc:
        with (
            tc.tile_pool(name="const", bufs=1) as constp,
            tc.tile_pool(name="pre", bufs=1) as prep,
            tc.tile_pool(name="kbuf", bufs=80) as kp,
            tc.tile_pool(name="scr", bufs=2) as scp,
            tc.tile_pool(name="pg", bufs=3, space="PSUM") as pg,
            tc.tile_pool(name="ps", bufs=3, space="PSUM") as ps,
        ):
            bias0 = constp.tile([128, 1], F32, tag="bias0")
            nc.vector.memset(bias0[:], 0.0)
            biasH = constp.tile([128, 1], F32, tag="biasH")
            nc.vector.memset(biasH[:], -LN2)

            # Warm-up exp: anchors the one-time ACT table load here (off the
            # critical path, and keeps later activations at <=2 sync waits).
            warm = constp.tile([128, 1], F32, tag="warm")
            nc.scalar.activation(warm[:], biasH[:], EXP, bias=bias0[:])

            abf = constp.tile([15, 2048], BF16)
            nc.sync.dma_start(abf[:], a_d.ap())
            bbf = constp.tile([15, 4096], BF16)
            nc.sync.dma_start(bbf[:], b_d.ap())

            # 2x2 pooling of seg (unscaled sum; the /4 per factor is folded
            # into the host-side constant). One contiguous load as [84, 4096]
            # (each partition holds 32 raw image rows of one class), pooling
            # via strided DVE adds entirely within partitions, then one
            # SBUF->SBUF DMA re-layout to the matmul layout [21, 4096].
            segfull = prep.tile([84, 4096], F32, tag="segfull")
            nc.sync.dma_start(segfull[:], seg_d.ap())
            sf = segfull[:].rearrange("p (y x) -> p y x", x=128)  # [84, 32, 128]
            ypool = prep.tile([84, 2048], F32, tag="ypool")
            yp = ypool[:].rearrange("p (y x) -> p y x", x=128)    # [84, 16, 128]
            nc.vector.tensor_add(yp, sf[:, 0::2, :], sf[:, 1::2, :])
            xin = ypool[:].rearrange("p (y x) -> p y x", x=2)     # [84, 1024, 2]
            pooled_flat = prep.tile([84, 1024], BF16, tag="poolf")
            nc.vector.tensor_add(pooled_flat[:], xin[:, :, 0], xin[:, :, 1])
            seg_sb = prep.tile([21, 4096], BF16, tag="segsb")
            nc.sync.dma_start(seg_sb[:], pooled_flat[:])

            accT = constp.tile([128, 80], F32)

            idx = 0
            for v in range(VB):
                base = v * 128
                ga = abf[:, base : base + 128]
                sa = seg_sb[:, base : base + 128]
                for c0, c1 in G_CHUNKS:
                    w = c1 - c0
                    pGt = pg.tile([128, 512], F32, tag="pg")
                    nc.tensor.matmul(
                        pGt[:, :w],
                        ga,
                        bbf[:, base + c0 : base + c1],
                        start=True,
                        stop=True,
                    )
                    kt = kp.tile([128, 512], BF16, tag="k")
                    if c0 == 0:
                        # d=0 (diagonal) block weighted 1/2 via exp bias
                        nc.scalar.activation(kt[:, 0:128], pGt[:, 0:128], EXP, bias=biasH[:])
                        nc.scalar.activation(kt[:, 128:512], pGt[:, 128:512], EXP, bias=bias0[:])
                    elif c0 == 2048:
                        # d=16 block weighted 1/2
                        nc.scalar.activation(kt[:, :w], pGt[:, :w], EXP, bias=biasH[:, :])
                    else:
                        nc.scalar.activation(kt[:, :w], pGt[:, :w], EXP, bias=bias0[:, :])
                    pSt = ps.tile([128, 512], F32, tag="ps")
                    nc.tensor.matmul(
                        pSt[:, :w],
                        sa,
                        seg_sb[:, base + c0 : base + c1],
                        start=True,
                        stop=True,
                    )
                    sct = scp.tile([128, 512], F32, tag="sc")
                    # out = (K mult 1.0) mult S; accum_out = sum(out) per row
                    nc.vector.scalar_tensor_tensor(
                        out=sct[:, :w],
                        in0=kt[:, :w],
                        scalar=1.0,
                        in1=pSt[:, :w],
                        op0=MULT,
                        op1=MULT,
                        accum_out=accT[:, idx : idx + 1],
                    )
                    idx += 1

            red = scp.tile([128, 1], F32, tag="red")
            nc.vector.tensor_reduce(
                red[:], accT[:], axis=mybir.AxisListType.X, op=ADD
            )
            nc.sync.dma_start(out_d.ap(), red[:])
    _split_multiwait(nc)
    return nc


def _split_multiwait(nc):
    """The walrus build here encodes at most one semaphore wait per compute
    instruction (setupSyncWait: 'Too many sync wait commands'). Tile emits
    multi-wait instructions, so hoist all but one wait onto standalone
    EventSemaphore instructions (what raw-bass wait_ge emits) placed just
    before the instruction on the same engine queue. Semantics identical:
    the engine blocks on each wait in turn."""
    ctr = 0
    for f in nc.m.functions:
        for blk in f.blocks:
            insts = blk.instructions
            out = []
            for inst in insts:
                si = inst.sync_info
                if si is not None and len(si.on_wait) > 1:
                    waits = list(si.on_wait)
                    for w in waits[:-1]:
                        es = mybir.InstEventSemaphore(
                            name=f"WSPLIT-{ctr}", ins=[], outs=[]
                        )
                        ctr += 1
                        es.engine = inst.engine
                        es.sync_info = bass_rust.SyncInfo(on_wait=[w], on_update=[])
                        out.append(es)
                    inst.sync_info = bass_rust.SyncInfo(
                        on_wait=[waits[-1]], on_update=list(si.on_update)
                    )
                out.append(inst)
            insts[:] = out


def _host_prep(images, segmentations):
    """Per-core inputs. Host work is reindexing (roll/stride/reshape) plus the
    tiny [5,4096] feature augmentation; all seg arithmetic happens on-device."""
    bf = ml_dtypes.bfloat16
    in_maps = []
    for c in range(8):
        n, h = c // 2, c % 2
        img = images[n][:, ::2, ::2]                       # nearest resize
        img = np.roll(img, -32 * h, axis=1).reshape(3, P)  # circulant rotation
        f = (img / SIGMA_RGB).astype(np.float32)
        f = f - f.mean(axis=1, keepdims=True)              # d2-invariant centering
        sq = (f * f).sum(axis=0)
        ones = np.ones((1, P), np.float32)
        b5 = np.concatenate([f, ones, (-0.5 * sq)[None]], axis=0)
        a5 = np.concatenate([f, (-0.5 * sq)[None], ones], axis=0)[:, : P // 2]

        def split(x):
            hi = x.astype(bf)
            lo = (x - hi.astype(np.float32)).astype(bf)
            return hi, lo

        a5h, a5l = split(a5)
        b5h, b5l = split(b5)
        abf = np.concatenate([a5h, a5l, a5h], axis=0)      # [15, 2048] bf16
        bbf = np.concatenate([b5h, b5h, b5l], axis=0)      # [15, 4096] bf16
        segr = np.roll(segmentations[n], -64 * h, axis=1)  # [21,128,128] f32
        in_maps.append(
            {
                "segr": np.ascontiguousarray(segr, dtype=np.float32),
                "abf": np.ascontiguousarray(abf),
                "bbf": np.ascontiguousarray(bbf),
            }
        )
    return in_maps


def run(images, segmentations, trace=False):
    if "nc" not in _CACHED:
        _CACHED["nc"] = _build_nc()
    nc = _CACHED["nc"]
    in_maps = _host_prep(np.asarray(images), np.asarray(segmentations))
    res = run_bass_kernel_spmd(nc, in_maps, list(range(8)), trace=trace)
    total = np.float64(0.0)
    for r in res.results:
        total += np.float64(r["acc"].astype(np.float64).sum())
    # x2 symmetric halves, /16 unscaled 2x2 pool (quadratic), -W, /N batch mean
    loss = -WEIGHT * 2.0 * total / 16.0 / N_IMG
    return np.array([loss], dtype=np.float32), res


def kernel(images, segmentations):
    out, _ = run(images, segmentations, trace=False)
    return out
